# revision 1
# baseline (speedup 1.0000x reference)
import os
import sys

import numpy as np

for _p in ("/opt/trn_rl_repo", "/root/.axon_site/_ro/trn_rl_repo"):
    if os.path.isdir(_p) and _p not in sys.path:
        sys.path.insert(0, _p)

# Problem constants (nn_CRF: feats [B,S,T] f32, masks [B,S] ones, transitions [T,T])
B, S, T = 512, 1024, 64
NC = 8            # cores
BL = B // NC      # 64 batches per core
NGRP = int(os.environ.get("CRF_NGRP", "3"))  # independent batch groups per core
RN_ENV = os.environ.get("CRF_RN")  # force renorm cadence (bench/debug)
LAG = 4           # renorm scale measured at app b is folded into g at app b+LAG
DBLK = 64         # time steps per DMA block
NBLK = S // DBLK  # 16
NEG = -10000.0
# g streaming dtype: bf16 (default) or fp8-e5m2 (halves HBM traffic; the
# ~7% quantization noise on g is far inside the error budget)
G_FP8 = bool(int(os.environ.get("CRF_G_FP8", "0")))

_CACHE = {}


def _build_bass(repeats=None, rn=10**9, ngrp=None):
    RN = rn
    # renorm needs 3 PSUM tiles per group; 8 banks cap that at 2 groups
    NG = ngrp if ngrp is not None else (NGRP if rn >= S else min(NGRP, 2))
    # group column offsets (uneven split allowed, e.g. NG=3 -> 22/21/21)
    GOFF = [round(gi * BL / NG) for gi in range(NG + 1)]
    BGS = [GOFF[i + 1] - GOFF[i] for i in range(NG)]
    import concourse.bacc as bacc
    import concourse.mybir as mybir
    from concourse.tile import TileContext
    import contextlib

    f32 = mybir.dt.float32
    bf16 = mybir.dt.bfloat16
    gdt = mybir.dt.float8e5 if G_FP8 else bf16
    Ln = mybir.ActivationFunctionType.Ln

    nc = bacc.Bacc()
    # g arranged host-side as [NBLK, T, DBLK, BL] so each DMA block is
    # contiguous per partition (DBLK*BL lines).
    g_in = nc.dram_tensor("g", [NBLK, T, DBLK, BL], gdt, kind="ExternalInput")
    # lhsT for the step matmul: [k, j] = exp(transitions[j, k])
    et_in = nc.dram_tensor("eaug", [T, T], bf16, kind="ExternalInput")
    xout = nc.dram_tensor("xout", [T, BL], bf16, kind="ExternalOutput")
    aux = nc.dram_tensor("aux", [2, BL], f32, kind="ExternalOutput")

    NX = int(os.environ.get("CRF_NX", "4"))  # X state rotation slots

    with TileContext(nc) as tc:
        with tc.tile_pool(name="const", bufs=1) as cpool, \
             tc.tile_pool(name="gp", bufs=3) as gpool, \
             tc.tile_pool(name="state", bufs=1) as xpool, \
             tc.tile_pool(name="ps", bufs=1, space="PSUM") as pspool, \
             tc.tile_pool(name="misc", bufs=2) as mpool:
            et_stage = cpool.tile([T, T], bf16)
            nc.sync.dma_start(et_stage, et_in[:, :])
            et = cpool.tile([T, T], bf16)
            # copy via DVE so matmuls depend only on the DVE semaphore
            nc.vector.tensor_copy(et, et_stage)
            ones_col = cpool.tile([T, 1], bf16)
            nc.vector.memset(ones_col, 1.0)
            ones_row = cpool.tile([1, T], bf16)
            nc.vector.memset(ones_row, 1.0)
            loop_cm = tc.For_i(0, repeats, 1) if repeats else contextlib.nullcontext()
            with loop_cm:
                xs, crows, pss, ps_ss, ps_bs, r_sbs, gs2s, lss = \
                    [], [], [], [], [], [], [], []
                for gi in range(NG):
                    rot = []
                    for sl_i in range(NX):
                        x_t = xpool.tile([T, BGS[gi]], bf16, name=f"x{gi}_{sl_i}",
                                         tag=f"x{gi}_{sl_i}")
                        rot.append(x_t)
                    xs.append(rot)
                    cr = xpool.tile([1, BGS[gi]], f32, name=f"c{gi}", tag=f"c{gi}")
                    nc.vector.memset(cr, 0.0)
                    crows.append(cr)
                    ps_t = pspool.tile([T, BGS[gi]], f32, name=f"ps{gi}", tag=f"ps{gi}")
                    pss.append(ps_t)
                    ps_s = pspool.tile([1, BGS[gi]], f32, name=f"pss{gi}", tag=f"pss{gi}")
                    ps_ss.append(ps_s)
                    if RN < S:
                        r_sb = xpool.tile([1, BGS[gi]], bf16, name=f"r{gi}",
                                          tag=f"r{gi}")
                        r_sbs.append(r_sb)
                        ps_b = pspool.tile([T, BGS[gi]], f32, name=f"psb{gi}",
                                           tag=f"psb{gi}")
                        ps_bs.append(ps_b)
                        gs2_t = xpool.tile([T, BGS[gi]], bf16, name=f"gs2{gi}",
                                           tag=f"gs2{gi}")
                        gs2s.append(gs2_t)
                    ls_t = xpool.tile([1, BGS[gi]], f32, name=f"ls{gi}", tag=f"ls{gi}")
                    lss.append(ls_t)
                # pending[gi] = app index whose g-slice must be scaled by 1/s
                pending = [None] * NG
                sb_last = [None] * NG
                app = 0
                for blk in range(NBLK):
                    gt = gpool.tile([T, DBLK, BL], gdt, tag="g")
                    nc.sync.dma_start(gt, g_in[blk])
                    for t in range(DBLK):
                        for gi in range(NG):
                            gsl = gt[:, t, GOFF[gi]:GOFF[gi + 1]]
                            if app == 0:
                                nc.vector.tensor_copy(xs[gi][0], gsl)
                                continue
                            xprev = xs[gi][(app - 1) % NX]
                            xcur = xs[gi][app % NX]
                            ps = pss[gi]
                            nc.tensor.matmul(ps, et, xprev, start=True, stop=True)
                            if pending[gi] == app:
                                # fold the pending 1/s renorm into this g slice
                                nc.vector.tensor_mul(gs2s[gi], gsl, ps_bs[gi])
                                gsl = gs2s[gi]
                                pending[gi] = None
                            nc.vector.tensor_mul(xcur, gsl, ps)
                            is_tap = (app % RN == RN - 1 and app + LAG <= S - 2)
                            if is_tap or app == S - 2:
                                # column sums of X_app via PE (ones^T @ X),
                                # broadcast back via PE (ones_row^T @ r)
                                nc.tensor.matmul(ps_ss[gi], ones_col, xcur,
                                                 start=True, stop=True)
                                ls = lss[gi]
                                nc.scalar.activation(ls, ps_ss[gi], Ln)
                                if is_tap:
                                    with nc.allow_low_precision(
                                            reason="1/s fold is compensated "
                                                   "exactly by the Ln term"):
                                        nc.vector.reciprocal(r_sbs[gi], ps_ss[gi])
                                    nc.tensor.matmul(ps_bs[gi], ones_row,
                                                     r_sbs[gi], start=True,
                                                     stop=True)
                                    nc.vector.tensor_add(crows[gi], crows[gi], ls)
                                    pending[gi] = app + LAG
                                if app == S - 2:
                                    sb_last[gi] = ls
                        app += 1
                for gi in range(NG):
                    cs = slice(GOFF[gi], GOFF[gi + 1])
                    nc.sync.dma_start(xout[:, cs], xs[gi][(S - 1) % NX])
                    nc.sync.dma_start(aux[0:1, cs], crows[gi])
                    nc.sync.dma_start(aux[1:2, cs], sb_last[gi])
    nc.finalize()
    return nc


def _drift_range(feats, transitions, F, nb=8):
    """ln X drift range over a batch subsample (float64 reference scan)."""
    idx = np.linspace(0, feats.shape[0] - 1, nb).astype(int)
    tr = transitions.astype(np.float64)
    f = feats[idx].astype(np.float64)
    alpha = f[:, 0]
    lo = hi = 0.0
    for i in range(1, f.shape[1]):
        sc = alpha[:, None, :] + tr[None] + f[:, i, :, None]
        m = sc.max(axis=2, keepdims=True)
        alpha = m[:, :, 0] + np.log(np.exp(sc - m).sum(axis=2))
        d = alpha[:, 1:] - (i + 1) * F
        lo = min(lo, float(d.min()))
        hi = max(hi, float(d.max()))
    return lo, hi


def _numpy_ref(feats, masks, transitions):
    # Exact log-domain fallback (only used if masks are not all ones).
    alpha = feats[:, 0].astype(np.float64)
    tr = transitions.astype(np.float64)
    for i in range(1, feats.shape[1]):
        sc = alpha[:, None, :] + tr[None] + feats[:, i, :, None].astype(np.float64)
        m = sc.max(axis=2, keepdims=True)
        new = (m[:, :, 0] + np.log(np.exp(sc - m).sum(axis=2)))
        mask = masks[:, i, None].astype(np.float64)
        alpha = new * mask + alpha * (1.0 - mask)
    return alpha.astype(np.float32)


def kernel(feats, masks, transitions):
    feats = np.asarray(feats, dtype=np.float32)
    masks = np.asarray(masks, dtype=np.float32)
    transitions = np.asarray(transitions, dtype=np.float32)
    if not np.all(masks == 1.0):
        return _numpy_ref(feats, masks, transitions)

    from concourse import bass_utils

    E = np.exp(transitions)                      # [j,k]; row/col 0 -> 0
    # growth-matched scale: per-step log-growth of x under the dynamics is
    # approximately ln(mean rowsum of E) + var(feat)/2 (lognormal mean of g).
    fm = float(feats.mean())
    fv = float(feats.var())
    F = float(np.log(E[1:, 1:].sum(axis=1).mean())) + fm + fv / 2.0
    if RN_ENV is not None:
        rn = int(RN_ENV)
    else:
        lo, hi = _drift_range(feats, transitions, F)
        # bf16 range is ~[-87, +88] nats; leave a wide margin
        rn = 10**9 if (lo > -45.0 and hi < 45.0) else 256
    key = ("nc", rn)
    if key not in _CACHE:
        _CACHE[key] = _build_bass(rn=rn)
    nc = _CACHE[key]
    eaug = np.ascontiguousarray(E.T)
    # g[b,s,j] = exp(feats - F) -> per-core [NBLK, T, DBLK, BL] bf16
    g = np.exp(feats - F)
    g = g.reshape(NC, BL, NBLK, DBLK, T).transpose(0, 2, 4, 3, 1)
    g = np.ascontiguousarray(g, dtype=np.float32)
    import ml_dtypes
    g16 = g.astype(ml_dtypes.float8_e5m2 if G_FP8 else ml_dtypes.bfloat16)

    in_maps = [{"g": g16[c], "eaug": eaug.astype(ml_dtypes.bfloat16)}
               for c in range(NC)]
    trace = bool(os.environ.get("CRF_TRACE"))
    import time as _time
    _t0 = _time.time()
    res = bass_utils.run_bass_kernel_spmd(
        nc, in_maps, core_ids=list(range(NC)), trace=trace)
    _CACHE.setdefault("t_run", []).append(_time.time() - _t0)
    _CACHE["last_res"] = res

    alpha = np.empty((B, T), np.float32)
    for c in range(NC):
        X = res.results[c]["xout"].astype(np.float64)    # [T, BL]
        crow = res.results[c]["aux"][0].astype(np.float64)   # [BL]
        lsl = res.results[c]["aux"][1].astype(np.float64)    # ln sum_k X_{S-2}
        a = np.log(np.maximum(X.T, 1e-300)) + (S * F + crow)[:, None]
        a[:, 0] = (feats[c * BL:(c + 1) * BL, S - 1, 0] + NEG
                   + lsl + (S - 1) * F + crow)
        alpha[c * BL:(c + 1) * BL] = a.astype(np.float32)
    return alpha

